# revision 1
# baseline (speedup 1.0000x reference)
"""Affinity-propagation spatial stencil kernel for Trainium2 (8 NeuronCores).

Data-parallel: 16 images sharded 2-per-core.

Math (A_k = zero-padded shift by OFFSETS[k]; G_k = guidance channel k):
  gate_k = A_k G_k;  absw = sum_k A_k |G_k|;  inv = 1/absw
  gate_sum = (sum_k A_k G_k) * inv;  bias = (1 - gate_sum) * raw
  step:  r' = inv * (sum_k A_k (G_k * r)) + bias
The shift of a product of shifts collapses: gate_k * A_k r = A_k (G_k * r),
so products are UNSHIFTED elementwise muls and only the sum needs shifts.
Shifted sums are grouped by row-shift class c in {-1,0,+1}:
  sum_k A_k x_k = sum_c rowshift_c( u_c ),  u_c = sum_{di_k=c} colshift_{dj_k} x_k
Column shifts are free-dim AP offsets (tiles carry zero guard columns); row
shifts are (partition,row) offsets with the partition-crossing row supplied
by a DMA-maintained halo plane (engine APs must be partition-aligned).

Layout: partition p holds image rows 4p..4p+3; free dim = [4 rows x W(+4)].

Engine split: DVE does the (aligned, 2x-mode-capable in fp16) products and
row-combines; GPSIMD does the column-shifted adds (no perf modes to lose) and
fp32->fp16 conversions; ACT does |G|, ln/exp (reciprocal) and the halo DMAs.
"""

import sys

sys.path.insert(0, "/opt/trn_rl_repo")

import numpy as np

import concourse.bass as bass
import concourse.mybir as mybir
from concourse import tile
from concourse.bass_utils import run_bass_kernel_spmd

N_CORES = 8
B, K, H, W = 16, 8, 512, 512
BPC = B // N_CORES  # images per core
P = 128
RPP = H // P  # rows per partition
WG = W + 4  # guarded row width (image cols at [2:514])
C0 = 2
PROP_TIME = 4
OFFSETS = ((1, 1), (1, 0), (1, -1), (0, 1), (0, -1), (-1, 1), (-1, 0), (-1, -1))

F32 = mybir.dt.float32
AT = mybir.AluOpType

DT = mybir.dt.float16  # compute dtype for gates/propagation
GP_ADDS = True  # run column-shifted adds on GPSIMD (else DVE)
SKIP_HALO = False  # timing experiment only: skip halo DMAs (numerically wrong)


def _split_excess_waits(nc):
    """This walrus build encodes at most 1 sem wait per instruction; move the
    overflow onto preceding NoOps. Also drop EVENT_SEMAPHORE_RANGE_CLEAR
    (unencodable here; only appears at the kernel tail where it's a no-op)."""
    for f in nc.m.functions:
        for bb in f.blocks:
            new_insts = []
            for ins in bb.instructions:
                if getattr(ins, "op_name", None) == "EVENT_SEMAPHORE_RANGE_CLEAR":
                    continue
                cap = 1
                si = getattr(ins, "sync_info", None)
                if si is not None and si.on_wait and len(si.on_wait) > cap:
                    extra = list(si.on_wait[cap:])
                    del si.on_wait[cap:]
                    while extra:
                        nop = mybir.InstNoOp(
                            name=nc.get_next_instruction_name(),
                            engine=ins.engine,
                            sync_info=mybir.SyncInfo(on_wait=extra[:cap], on_update=[]),
                        )
                        new_insts.append(nop)
                        extra = extra[cap:]
                new_insts.append(ins)
            bb.instructions[:] = new_insts


def _c(ap):
    """center (image) view of a guarded [P, RPP, WG] tile."""
    return ap[:, :, C0 : C0 + W]


def _w(ap, dj):
    """column-shifted view of a guarded tile: value at [i, j+dj]."""
    return ap[:, :, C0 + dj : C0 + dj + W]


def _emit_combine(nc, V, acc_rows, up, u0, um, halo_dn, halo_up):
    """acc[row] = up[row+1] + um[row-1] + u0[row] with zero pad, then the
    caller's finisher runs on acc. halo_dn[p] = up[row 4p+4], halo_up[p] =
    um[row 4p-1] (rows outside the image stay zero)."""
    if not SKIP_HALO:
        nc.scalar.dma_start(out=halo_dn[0 : P - 1, 0:1, :], in_=up[1:P, 0:1, :])
        nc.scalar.dma_start(out=halo_up[1:P, 0:1, :], in_=um[0 : P - 1, 3:4, :])
    V.tensor_add(acc_rows[:, 1:3, :], up[:, 2:4, :], um[:, 0:2, :])
    V.tensor_add(acc_rows[:, 0:1, :], up[:, 1:2, :], halo_up[:, 0:1, :])
    V.tensor_add(acc_rows[:, 3:4, :], halo_dn[:, 0:1, :], um[:, 2:3, :])


def _emit_image(nc, pool, g_dram, d_dram, o_dram, b):
    V = nc.vector
    GP = nc.gpsimd if GP_ADDS else nc.vector
    GPR = nc.gpsimd
    ABS = mybir.ActivationFunctionType.Abs
    fp16 = DT != F32

    def wtile(name, dtype=None, bufs=None):
        return pool.tile([P, RPP, W], dtype or DT, name=name, bufs=bufs)

    gates = pool.tile([P, K, RPP, WG], DT, name="gates", bufs=2)
    ta = pool.tile([P, RPP, WG], DT, name="ta")
    tb = pool.tile([P, RPP, WG], DT, name="tb")
    tc_ = pool.tile([P, RPP, WG], DT, name="tc_")
    td = pool.tile([P, RPP, WG], DT, name="td")
    up = wtile("up")
    u0 = wtile("u0")
    um = wtile("um")
    acc = wtile("acc")
    r0 = wtile("r0", bufs=2)
    r1 = wtile("r1")
    inv = wtile("inv")
    bias = wtile("bias")
    absw = wtile("absw")
    lnw = wtile("lnw", F32)
    res = wtile("res", F32)
    halo_dn = pool.tile([P, 1, W], DT, name="halo_dn")
    halo_up = pool.tile([P, 1, W], DT, name="halo_up")
    if fp16:
        stage = wtile("stage", F32, bufs=2)

    # Zero pads: guard columns of guarded tiles; halo rows outside the image.
    GPR.memset(gates[:, :, :, 0:C0], 0.0)
    GPR.memset(gates[:, :, :, C0 + W : WG], 0.0)
    for t in (ta, tb, tc_, td):
        GPR.memset(t[:, :, 0:C0], 0.0)
        GPR.memset(t[:, :, C0 + W : WG], 0.0)
    GPR.memset(halo_dn[:], 0.0)
    GPR.memset(halo_up[:], 0.0)

    for k in range(K):
        src = g_dram[b, k].rearrange("(p r) j -> p r j", p=P)
        if fp16:
            nc.sync.dma_start(out=stage[:], in_=src)
            nc.vector.tensor_copy(_c(gates[:, k]), stage[:])
        else:
            nc.sync.dma_start(out=_c(gates[:, k]), in_=src)
    dsrc = d_dram[b, 0].rearrange("(p r) j -> p r j", p=P)
    if fp16:
        nc.sync.dma_start(out=stage[:], in_=dsrc)
        nc.vector.tensor_copy(r0[:], stage[:])
    else:
        nc.sync.dma_start(out=r0[:], in_=dsrc)

    def gv(k):  # guarded gate plane view [P, RPP, WG]
        return gates[:, k]

    # ---- absw = sum_k A_k |G_k| ----
    nc.scalar.activation(_c(ta), _c(gv(0)), ABS)
    nc.scalar.activation(_c(tb), _c(gv(1)), ABS)
    V.tensor_add(up[:], _w(ta, 1), _c(tb))
    nc.scalar.activation(_c(tc_), _c(gv(2)), ABS)
    V.tensor_add(up[:], up[:], _w(tc_, -1))
    nc.scalar.activation(_c(td), _c(gv(3)), ABS)
    nc.scalar.activation(_c(tc_), _c(gv(4)), ABS)
    V.tensor_add(u0[:], _w(td, 1), _w(tc_, -1))
    nc.scalar.activation(_c(ta), _c(gv(5)), ABS)
    nc.scalar.activation(_c(tb), _c(gv(6)), ABS)
    GP.tensor_add(um[:], _w(ta, 1), _c(tb))
    nc.scalar.activation(_c(ta), _c(gv(7)), ABS)
    GP.tensor_add(um[:], um[:], _w(ta, -1))
    _emit_combine(nc, V, absw, up, u0, um, halo_dn, halo_up)
    V.tensor_add(absw[:], absw[:], u0[:])
    # inv = exp(-ln(absw)) on the scalar engine
    nc.scalar.activation(lnw[:], absw[:], mybir.ActivationFunctionType.Ln)
    nc.scalar.activation(inv[:], lnw[:], mybir.ActivationFunctionType.Exp, scale=-1.0)

    # ---- bias = (1 - inv * sum_k A_k G_k) * raw ----
    V.tensor_add(up[:], _w(gv(0), 1), _c(gv(1)))
    V.tensor_add(up[:], up[:], _w(gv(2), -1))
    V.tensor_add(u0[:], _w(gv(3), 1), _w(gv(4), -1))
    GP.tensor_add(um[:], _w(gv(5), 1), _c(gv(6)))
    GP.tensor_add(um[:], um[:], _w(gv(7), -1))
    _emit_combine(nc, V, acc, up, u0, um, halo_dn, halo_up)
    V.tensor_add(acc[:], acc[:], u0[:])
    V.tensor_mul(acc[:], acc[:], inv[:])  # gate_sum
    V.tensor_mul(acc[:], acc[:], r0[:])  # gate_sum * raw
    V.tensor_sub(bias[:], r0[:], acc[:])

    # ---- propagation ----
    cur, nxt = r0, r1
    for step in range(PROP_TIME):
        V.tensor_mul(_c(ta), _c(gv(0)), cur[:])
        V.tensor_mul(_c(tb), _c(gv(1)), cur[:])
        V.tensor_mul(_c(tc_), _c(gv(2)), cur[:])
        V.tensor_add(up[:], _w(ta, 1), _c(tb))
        V.tensor_add(up[:], up[:], _w(tc_, -1))
        V.tensor_mul(_c(ta), _c(gv(3)), cur[:])
        V.tensor_mul(_c(tb), _c(gv(4)), cur[:])
        V.tensor_add(u0[:], _w(ta, 1), _w(tb, -1))
        V.tensor_mul(_c(tc_), _c(gv(5)), cur[:])
        V.tensor_mul(_c(td), _c(gv(6)), cur[:])
        GP.tensor_add(um[:], _w(tc_, 1), _c(td))
        V.tensor_mul(_c(td), _c(gv(7)), cur[:])
        GP.tensor_add(um[:], um[:], _w(td, -1))
        _emit_combine(nc, V, acc, up, u0, um, halo_dn, halo_up)
        V.tensor_add(acc[:], acc[:], u0[:])
        if step < PROP_TIME - 1:
            V.tensor_mul(acc[:], acc[:], inv[:])
            V.tensor_add(nxt[:], acc[:], bias[:])
            cur, nxt = nxt, cur
        else:
            V.tensor_mul(res[:], acc[:], inv[:])
            V.tensor_add(res[:], res[:], bias[:])

    nc.sync.dma_start(
        out=o_dram[b, 0].rearrange("(p r) j -> p r j", p=P), in_=res[:]
    )


def build(legalize=True, repeat=1):
    nc = bass.Bass()
    g_dram = nc.declare_dram_parameter("guidance", [BPC, K, H, W], F32, isOutput=False)
    d_dram = nc.declare_dram_parameter("blur_depth", [BPC, 1, H, W], F32, isOutput=False)
    o_dram = nc.declare_dram_parameter("out", [BPC, 1, H, W], F32, isOutput=True)
    with tile.TileContext(nc) as tc:
        with tc.tile_pool(name="main", bufs=1) as pool:
            for _ in range(repeat):
                for b in range(BPC):
                    _emit_image(nc, pool, g_dram, d_dram, o_dram, b)
    if legalize:
        _split_excess_waits(nc)
    return nc


_NC = None


def _get_nc():
    global _NC
    if _NC is None:
        _NC = build()
    return _NC


def run(guidance, blur_depth, **spmd_kwargs):
    nc = _get_nc()
    in_maps = [
        {
            "guidance": np.ascontiguousarray(guidance[BPC * c : BPC * (c + 1)]),
            "blur_depth": np.ascontiguousarray(blur_depth[BPC * c : BPC * (c + 1)]),
        }
        for c in range(N_CORES)
    ]
    res = run_bass_kernel_spmd(nc, in_maps, list(range(N_CORES)), **spmd_kwargs)
    out = np.concatenate([res.results[i]["out"] for i in range(N_CORES)], axis=0)
    return out, res


def kernel(guidance, blur_depth):
    out, _ = run(guidance, blur_depth)
    return out.astype(np.float32)

